# revision 73
# baseline (speedup 1.0000x reference)
"""DSS Linear+BN segment-reduce kernel for Trainium2, 8 NeuronCores.

Problem (N=131072, D=1024, B=2048):
    z_i = BN(x @ W_fc.T + b_fc)                      # per-element path
    x_m = segment_sum(x, seg_ids, B)                 # ragged segment sums
    x_s = BN(x_m @ W_sh.T + b_sh)                    # set path
    out = x_s[seg_ids] + z_i

Strategy (segment-aligned data parallel, one collective):
  - Host shards rows by whole segments: exactly 256 segments per core
    (greedy balance on padded length). Rows laid out transposed (k on
    partitions), each segment zero-padded to a multiple of 8. Within each
    1024-col block, columns are permuted j-major (col = j*128 + c for
    within-chunk j, chunk c) so chunk-8 partial sums become 3 contiguous
    tree adds on DVE. BN biases are absorbed into the BN shift.
  - Phase A: z'^T = W^T.T @ x^T in bf16 (1024-col passes, f32 psum), spilled
    to DRAM bf16; bn_stats per tile; chunk sums tree-added and PE-transposed
    into an SBUF-resident chunk table S [128, nblk, 1024].
  - Segment reduce: x_m^T = S-tiles.T @ onehot(chunk->seg) via PE matmuls
    (host-precomputed one-hot tiles). Set-path matmul + bn_stats follow.
  - One AllGather of packed BN sums [128,32]; exact on-chip rank reduce.
    Combined table C = x_s + t_fc written to DRAM bf16 [256, 1024].
  - Pass 2: out^T = z'^T * s_fc + Cwin^T @ onehot(sid_rel), where each
    1024-col block's segments fit a 128-row window of C (host asserts),
    gathered by indirect DMA; one matmul per (dc, block). Output bf16.
  - Host transposes/gathers real rows back to [N, 1024] f32.
"""
import sys
import numpy as np
from contextlib import ExitStack

sys.path.insert(0, "/opt/trn_rl_repo")

import concourse.bass as bass
import concourse.bacc as bacc
import concourse.tile as tile
from concourse import mybir
from concourse.bass_utils import run_bass_kernel_spmd

F32 = mybir.dt.float32
BF16 = mybir.dt.bfloat16
I32 = mybir.dt.int32
AX = mybir.AxisListType.X
ALU = mybir.AluOpType

N, D, B, NC = 131072, 1024, 2048, 8
B_PER = B // NC            # 256 segments per core
EPS = 1e-5
CH = 8                     # segment padding / chunk size
RB = 1024                  # rows per block (matmul free dim)
CPB = RB // CH             # 128 chunks per block
KC = D // 128              # 8 k-chunks
DC = D // 128              # 8 d-chunks

_cache = {}


def _plan(seg_ids):
    """Host planning: per-core padded, block-permuted layouts."""
    seg_ids = np.asarray(seg_ids)
    counts = np.bincount(seg_ids, minlength=B).astype(np.int64)
    row_start = np.zeros(B + 1, dtype=np.int64)
    np.cumsum(counts, out=row_start[1:])

    pad = ((counts + CH - 1) // CH) * CH          # padded len per segment
    # Balanced assignment: exactly B_PER segments per core (program-uniform),
    # greedily packing large segments onto the least-loaded core.
    order = np.argsort(-pad, kind="stable")
    load = np.zeros(NC, dtype=np.int64)
    nseg = np.zeros(NC, dtype=np.int64)
    assign = np.empty(B, dtype=np.int64)
    for b in order:
        cands = np.where(nseg < B_PER)[0]
        c = cands[np.argmin(load[cands])]
        assign[b] = c
        load[c] += pad[b]
        nseg[c] += 1
    max_rows = int(((load.max() + RB - 1) // RB) * RB)
    nblk = max_rows // RB
    nch_tot = max_rows // CH                      # == nblk * CPB

    # j-major permutation within each block: unpermuted padded position i
    # (chunk c, within j) -> block*RB + j*CPB + c
    i = np.arange(max_rows, dtype=np.int64)
    blk, w = i // RB, i % RB
    perm = blk * RB + (w % CH) * CPB + (w // CH)

    plans = []
    for c in range(NC):
        segs = np.where(assign == c)[0]          # global segment ids, sorted
        cnt = counts[segs]
        pd = pad[segs]
        pstart = np.zeros(B_PER, dtype=np.int64)
        np.cumsum(pd[:-1], out=pstart[1:])
        nreal = int(cnt.sum())
        gr = np.concatenate(
            [np.arange(row_start[b], row_start[b + 1]) for b in segs]) \
            if nreal else np.empty(0, dtype=np.int64)
        local_b = np.repeat(np.arange(B_PER), cnt)
        col_u = np.repeat(pstart, cnt) + \
            (np.arange(nreal) - np.repeat(np.cumsum(cnt) - cnt, cnt))
        col_p = perm[col_u]                      # permuted final columns

        # local seg id per unpermuted padded column (999 on pad cols)
        sid_u = np.full(max_rows, 999.0, dtype=np.float64)
        sid_u[col_u] = local_b
        # also padded-but-in-segment cols carry their segment (harmless
        # either way since their x is 0; keep 999 so e-matches are sparse)

        # per-block window base over REAL cols; sid_rel relative to it
        sid_p = np.full(max_rows, 999.0)
        sid_p[perm] = sid_u
        wb = np.zeros(nblk, dtype=np.int64)
        for ib in range(nblk):
            s = sid_p[ib * RB:(ib + 1) * RB]
            s = s[s < 999]
            if s.size:
                lo, hi = int(s.min()), int(s.max())
                assert hi - lo <= 127, f"window overflow core{c} blk{ib}"
                wb[ib] = min(lo, B_PER - 128)
        sid_rel = sid_p.copy()
        m = sid_p < 999
        sid_rel[m] = sid_p[m] - wb[np.nonzero(m)[0] // RB]
        widx = (wb[None, :] + np.arange(128)[:, None]).astype(np.int32)

        # one-hot chunk->local-seg tiles [nblk, 128, B_PER]
        nch = (pd // CH).astype(np.int64)
        cstart = pstart // CH
        oh = np.zeros((nch_tot, B_PER), dtype=np.float64)
        for b in range(B_PER):
            oh[cstart[b]:cstart[b] + nch[b], b] = 1.0
        oh = oh.reshape(nblk, CPB, B_PER)

        # one-hot window rows per block: e[ib, p, r] = (sid_rel == p)
        sr = sid_rel.reshape(nblk, RB)
        ein = (sr[:, None, :] == np.arange(128.0)[None, :, None])
        plans.append(dict(grows=gr, col_p=col_p, ein=ein,
                          widx=widx, oh=oh))
    return plans, max_rows


def _build(max_rows):
    nblk = max_rows // RB

    nc = bacc.Bacc("TRN2", target_bir_lowering=False, debug=False,
                   num_devices=NC)

    # block-major: [block, partition, kc, r], 16KB contiguous per partition
    xT = nc.dram_tensor("xT", [nblk * 128, KC * RB], BF16,
                        kind="ExternalInput").ap()
    ein = nc.dram_tensor("ein", [nblk * 128, RB], BF16,
                         kind="ExternalInput").ap()
    widx = nc.dram_tensor("widx", [128, nblk], I32, kind="ExternalInput").ap()
    oh = nc.dram_tensor("oh", [nblk * 128, B_PER], BF16,
                        kind="ExternalInput").ap()
    wfT = nc.dram_tensor("wfT", [D, D], BF16, kind="ExternalInput").ap()
    wsT = nc.dram_tensor("wsT", [D, D], BF16, kind="ExternalInput").ap()
    # params [128, 162]: 0:8 g_fc, 8:16 be_fc, 16:24 g_sh, 24:32 be_sh,
    #                    32 iota, 34:162 identity
    par = nc.dram_tensor("par", [128, 162], F32, kind="ExternalInput").ap()
    # block-major: [block, partition, dc, r] so each partition's slice of a
    # block is one contiguous 16KB DMA descriptor (8x fewer descriptors)
    outT = nc.dram_tensor("outT", [nblk * 128, DC * RB], BF16,
                          kind="ExternalOutput").ap()

    wfT3 = wfT.rearrange("(kc p) d -> p kc d", p=128)
    wsT3 = wsT.rearrange("(kc p) d -> p kc d", p=128)

    with tile.TileContext(nc) as tc:
        with ExitStack() as top:
            keep = top.enter_context(tc.tile_pool(name="keep", bufs=1))
            dram = top.enter_context(tc.tile_pool(name="dram", bufs=1,
                                                  space="DRAM"))

            zT = dram.tile([nblk * 128, DC * RB], BF16)
            ctab = dram.tile([B_PER, D], BF16)
            d_st_in = dram.tile([128, 32], F32)
            d_st_ag = dram.tile([NC, 128, 32], F32)

            p_par = keep.tile([128, 162], F32)
            nc.sync.dma_start(p_par[:], par[:])
            p_widx = keep.tile([128, nblk], I32)
            nc.sync.dma_start(p_widx[:], widx[:])
            # off the sync queue so it doesn't delay the first x/w loads
            p_oh = keep.tile([128, nblk, B_PER], BF16)
            nc.gpsimd.dma_start(p_oh[:],
                                oh.rearrange("(t p) b -> p t b", p=128))
            ident = p_par[:, 34:162]
            identb = keep.tile([128, 128], BF16)
            nc.vector.tensor_copy(identb[:], ident)

            S = keep.tile([128, nblk, D], BF16)         # chunk sums
            bn_i = keep.tile([128, DC, 2 * nblk, 6], F32)  # element stats
            s_fc = keep.tile([128, DC], F32)            # element-path scale

            # z tiles for the last HOLD blocks stay in SBUF (no spill);
            # pool must outlive phase A, so it lives in the outer stack.
            HOLD = 2
            zpool = top.enter_context(tc.tile_pool(name="za", bufs=2))
            zheld = {}
            # top-level so its SBUF range is disjoint from middle-phase
            # pools: zt prefetches then start during the collective instead
            # of blocking on middle-pool SBUF reuse (WAR).
            z2pool = top.enter_context(tc.tile_pool(name="z2", bufs=3))

            # ============ PHASE A ============
            with ExitStack() as pa:
                wpool = pa.enter_context(tc.tile_pool(name="wf", bufs=1))
                xpool = pa.enter_context(tc.tile_pool(name="xa", bufs=2))
                spool = pa.enter_context(tc.tile_pool(name="sa", bufs=1))
                s8pool = pa.enter_context(tc.tile_pool(name="s8", bufs=2))
                psA = pa.enter_context(
                    tc.tile_pool(name="psA", bufs=3, space="PSUM"))
                psT = pa.enter_context(
                    tc.tile_pool(name="psT", bufs=2, space="PSUM"))

                wf = wpool.tile([128, KC, D], BF16)
                xt0 = xpool.tile([128, KC, RB], BF16, tag="xt")
                nc.sync.dma_start(xt0[:].rearrange("p kc r -> p (kc r)"),
                                  xT[0:128, :])
                nc.sync.dma_start(wf[:], wfT3)

                for ib in range(nblk):
                    if ib == 0:
                        xt = xt0
                    else:
                        xt = xpool.tile([128, KC, RB], BF16, tag="xt")
                        nc.sync.dma_start(
                            xt[:].rearrange("p kc r -> p (kc r)"),
                            xT[ib * 128:(ib + 1) * 128, :])
                    zst = zpool.tile([128, DC, RB], BF16, tag="zst")
                    s8b = None
                    for dc in range(DC):
                        pz = psA.tile([128, RB], F32, tag="mm")
                        for kc in range(KC):
                            for q in range(2):
                                qs = slice(q * (RB // 2), (q + 1) * (RB // 2))
                                nc.tensor.matmul(
                                    pz[:, qs],
                                    wf[:, kc, dc * 128:(dc + 1) * 128],
                                    xt[:, kc, qs], start=(kc == 0),
                                    stop=(kc == KC - 1))
                        if dc == 3:
                            # chunk-8 tree mid-way through the dc loop: s8b
                            # is then ready before the PE reaches this
                            # block's transposes (no block-boundary stall),
                            # without delaying the first bn_stats.
                            xt4 = xt[:].rearrange("p kc (j c) -> p kc j c",
                                                  j=CH)
                            t4 = spool.tile([128, KC, 4, CPB], BF16,
                                            tag="t4")
                            nc.vector.tensor_add(t4[:], xt4[:, :, 0:4, :],
                                                 xt4[:, :, 4:8, :])
                            t2 = spool.tile([128, KC, 2, CPB], BF16,
                                            tag="t2")
                            nc.vector.tensor_add(t2[:], t4[:, :, 0:2, :],
                                                 t4[:, :, 2:4, :])
                            s8b = s8pool.tile([128, KC, CPB], BF16,
                                              tag="s8b")
                            nc.vector.tensor_add(s8b[:], t2[:, :, 0, :],
                                                 t2[:, :, 1, :])
                        nc.vector.bn_stats(bn_i[:, dc, 2 * ib, :],
                                           pz[:, 0:RB // 2])
                        nc.vector.bn_stats(bn_i[:, dc, 2 * ib + 1, :],
                                           pz[:, RB // 2:RB])
                        nc.scalar.copy(zst[:, dc, :], pz[:])
                    if ib >= nblk - HOLD:
                        zheld[ib] = zst
                    else:
                        nc.sync.dma_start(
                            zT[ib * 128:(ib + 1) * 128, :],
                            zst[:].rearrange("p dc r -> p (dc r)"))

                    for kc in range(KC):
                        pt = psT.tile([CPB, 128], BF16, tag="tr")
                        nc.tensor.transpose(pt[:], s8b[:, kc, :], identb[:])
                        nc.vector.tensor_copy(
                            S[:, ib, kc * 128:(kc + 1) * 128], pt[:])

            # ============ SEGMENT REDUCE + SET PATH + STATS ============
            with ExitStack() as pm:
                wpool2 = pm.enter_context(tc.tile_pool(name="ws", bufs=1))
                mpool = pm.enter_context(tc.tile_pool(name="mid", bufs=1))

                ws = wpool2.tile([128, KC, D], BF16)
                nc.sync.dma_start(ws[:], wsT3)

                # prefetch the first pass-2 z tiles NOW: their deps resolved
                # long ago, and anything issued later on this queue may sit
                # behind collective-linked waits.
                zpre = {}
                for ib in range(3):
                    zp = z2pool.tile([128, DC, RB], BF16, tag="zt",
                                     name=f"zpre{ib}")
                    nc.sync.dma_start(
                        zp[:].rearrange("p dc r -> p (dc r)"),
                        zT[ib * 128:(ib + 1) * 128, :])
                    zpre[ib] = zp

                loc = mpool.tile([128, 32], F32)

                def pack16(mv, cnt_, loc_sl):
                    nc.vector.tensor_scalar_mul(loc[:, loc_sl][:, 0:8],
                                                mv[:, :, 0], cnt_)
                    tq = mpool.tile([128, DC], F32, tag="tq")
                    nc.vector.tensor_mul(tq[:], mv[:, :, 0], mv[:, :, 0])
                    nc.vector.tensor_add(tq[:], tq[:], mv[:, :, 1])
                    nc.vector.tensor_scalar_mul(loc[:, loc_sl][:, 8:16],
                                                tq[:], cnt_)

                # element-path stats are complete at phase-A end: pack them
                # now so DVE work overlaps the segment reduce on the PE.
                mv_i = mpool.tile([128, DC, 2], F32)
                for dc in range(DC):
                    nc.vector.bn_aggr(mv_i[:, dc, :], bn_i[:, dc, :, :])
                pack16(mv_i, float(max_rows), slice(0, 16))

                # x_m^T[k, b] = sum_t S[:, t, k].T @ onehot[t]
                xmT = mpool.tile([128, KC, B_PER], BF16)
                with ExitStack() as ps_a:
                    psM = ps_a.enter_context(
                        tc.tile_pool(name="psM", bufs=1, space="PSUM"))
                    pxm = [psM.tile([128, B_PER], F32, tag=f"xm{kc}",
                                    name=f"pxm{kc}")
                           for kc in range(KC)]
                    for t in range(nblk):
                        for kc in range(KC):
                            nc.tensor.matmul(
                                pxm[kc][:],
                                S[:, t, kc * 128:(kc + 1) * 128],
                                p_oh[:, t, :], start=(t == 0),
                                stop=(t == nblk - 1))
                    for kc in range(KC):
                        nc.vector.tensor_copy(xmT[:, kc, :], pxm[kc][:])

                # set path: z_s^T[d, b]
                zsT = mpool.tile([128, DC, B_PER], F32)
                bn_s = mpool.tile([128, DC, 1, 6], F32)
                with ExitStack() as ps_b:
                    psS = ps_b.enter_context(
                        tc.tile_pool(name="psS", bufs=1, space="PSUM"))
                    for dc in range(DC):
                        pzs = psS.tile([128, B_PER], F32, tag=f"s{dc}")
                        for kc in range(KC):
                            nc.tensor.matmul(
                                pzs[:], ws[:, kc, dc * 128:(dc + 1) * 128],
                                xmT[:, kc, :], start=(kc == 0),
                                stop=(kc == KC - 1))
                        nc.vector.bn_stats(bn_s[:, dc, 0, :], pzs[:])
                        nc.vector.tensor_copy(zsT[:, dc, :], pzs[:])

                # set-path stats pack, then the single collective
                mv_s = mpool.tile([128, DC, 2], F32)
                for dc in range(DC):
                    nc.vector.bn_aggr(mv_s[:, dc, :], bn_s[:, dc, :, :])
                pack16(mv_s, float(B_PER), slice(16, 32))
                nc.sync.dma_start(d_st_in[:], loc[:])
                nc.gpsimd.collective_compute(
                    "AllGather", ALU.bypass,
                    replica_groups=[list(range(NC))],
                    ins=[d_st_in[:].opt()], outs=[d_st_ag[:].opt()])

                # rk load waits on the collective: keep it OFF the sync
                # DMA queue so pass-2 prefetches are not head-blocked.
                rk = mpool.tile([128, NC, 32], F32)
                nc.scalar.dma_start(rk[:], d_st_ag.rearrange("r p j -> p r j"))
                g32 = mpool.tile([128, 32], F32)
                nc.vector.reduce_sum(out=g32[:],
                                     in_=rk[:].rearrange("p r j -> p j r"),
                                     axis=AX)

                # scales/shifts per d-column ([128, 8] transposed layout)
                def bn_affine(g_off, inv_n, g_sl, be_sl, s_out, sfx):
                    m = mpool.tile([128, DC], F32, tag=f"m{sfx}")
                    nc.vector.tensor_scalar_mul(m[:], g32[:, g_off:g_off + 8],
                                                inv_n)
                    v = mpool.tile([128, DC], F32, tag=f"v{sfx}")
                    nc.vector.tensor_scalar_mul(
                        v[:], g32[:, g_off + 8:g_off + 16], inv_n)
                    t2_ = mpool.tile([128, DC], F32, tag=f"t2{sfx}")
                    nc.vector.tensor_mul(t2_[:], m[:], m[:])
                    nc.vector.tensor_sub(v[:], v[:], t2_[:])
                    nc.vector.tensor_scalar_add(v[:], v[:], EPS)
                    nc.scalar.sqrt(v[:], v[:])
                    nc.vector.reciprocal(v[:], v[:])
                    nc.vector.tensor_mul(s_out[:], v[:], p_par[:, g_sl])
                    t_out = mpool.tile([128, DC], F32, tag=f"t{sfx}")
                    nc.vector.tensor_mul(t_out[:], m[:], s_out[:])
                    nc.vector.tensor_sub(t_out[:], p_par[:, be_sl], t_out[:])
                    return t_out

                t_fc = bn_affine(0, 1.0 / N,
                                 slice(0, 8), slice(8, 16), s_fc, "i")
                s_sh = mpool.tile([128, DC], F32)
                t_sh = bn_affine(16, 1.0 / B,
                                 slice(16, 24), slice(24, 32), s_sh, "s")
                tb = mpool.tile([128, DC], F32)
                nc.vector.tensor_add(tb[:], t_sh[:], t_fc[:])

                # C^T = z_s^T * s_sh + (t_sh + t_fc); transpose -> DRAM table
                ct = mpool.tile([128, DC, B_PER], F32)
                for dc in range(DC):
                    nc.vector.tensor_scalar(
                        out=ct[:, dc, :], in0=zsT[:, dc, :],
                        scalar1=s_sh[:, dc:dc + 1], scalar2=tb[:, dc:dc + 1],
                        op0=ALU.mult, op1=ALU.add)
                cnat = mpool.tile([128, 2, D], BF16)
                with ExitStack() as ps_c:
                    psC = ps_c.enter_context(
                        tc.tile_pool(name="psC", bufs=2, space="PSUM"))
                    for h in range(2):
                        for dc in range(DC):
                            pt = psC.tile([128, 128], F32, tag="tr")
                            nc.tensor.transpose(
                                pt[:], ct[:, dc, h * 128:(h + 1) * 128],
                                ident)
                            nc.vector.tensor_copy(
                                cnat[:, h, dc * 128:(dc + 1) * 128], pt[:])
                # gpsimd queue: chains straight into the pass-2 cw gathers
                for h in range(2):
                    nc.gpsimd.dma_start(ctab[h * 128:(h + 1) * 128, :],
                                        cnat[:, h, :])

            # ============ PASS 2 ============
            with ExitStack() as p2:
                opool = p2.enter_context(tc.tile_pool(name="o2", bufs=3))
                cwpool = p2.enter_context(tc.tile_pool(name="cw", bufs=3))
                epool = p2.enter_context(tc.tile_pool(name="ep", bufs=3))
                # held blocks first: their z is already in SBUF, so their
                # compute fills the pipeline while zt loads stream.
                border = [nblk - 2, nblk - 1] + list(range(nblk - 2))
                epre = {}
                for ib in border[:3]:
                    ep = epool.tile([128, RB], BF16, tag="et",
                                    name=f"epre{ib}")
                    nc.gpsimd.dma_start(ep[:],
                                        ein[ib * 128:(ib + 1) * 128, :])
                    epre[ib] = ep
                tpool = p2.enter_context(tc.tile_pool(name="tp", bufs=2))
                ps2 = p2.enter_context(
                    tc.tile_pool(name="ps2", bufs=4, space="PSUM"))

                for ib in border:
                    if ib in zheld:
                        zt = zheld[ib]
                    elif ib in zpre:
                        zt = zpre[ib]
                    else:
                        zt = z2pool.tile([128, DC, RB], BF16, tag="zt")
                        nc.sync.dma_start(
                            zt[:].rearrange("p dc r -> p (dc r)"),
                            zT[ib * 128:(ib + 1) * 128, :])
                    if ib in epre:
                        et = epre[ib]
                    else:
                        et = epool.tile([128, RB], BF16, tag="et")
                        nc.gpsimd.dma_start(et[:],
                                            ein[ib * 128:(ib + 1) * 128, :])
                    cw = cwpool.tile([128, D], BF16, tag="cw")
                    nc.gpsimd.indirect_dma_start(
                        out=cw[:],
                        out_offset=None,
                        in_=ctab[:, :],
                        in_offset=bass.IndirectOffsetOnAxis(
                            ap=p_widx[:, ib:ib + 1], axis=0),
                        element_offset=0)
                    ob = opool.tile([128, DC, RB], BF16, tag="ob")
                    for dc in range(DC):
                        px = ps2.tile([128, RB], F32, tag="ex")
                        for q in range(2):
                            qs = slice(q * (RB // 2), (q + 1) * (RB // 2))
                            nc.tensor.matmul(
                                px[:, qs], cw[:, dc * 128:(dc + 1) * 128],
                                et[:, qs], start=True, stop=True)
                        tm = tpool.tile([128, RB], BF16, tag=f"tm{dc % 2}")
                        if dc % 2 == 0:
                            nc.vector.tensor_scalar_mul(
                                tm[:], zt[:, dc, :], s_fc[:, dc:dc + 1])
                        else:
                            nc.scalar.mul(tm[:], zt[:, dc, :],
                                          s_fc[:, dc:dc + 1])
                        nc.vector.tensor_add(ob[:, dc, :], tm[:], px[:])
                    nc.sync.dma_start(
                        outT[ib * 128:(ib + 1) * 128, :],
                        ob[:].rearrange("p dc r -> p (dc r)"))

    nc.compile()
    return nc


def kernel(x, W_fc, b_fc, g_fc, be_fc, W_sh, b_sh, g_sh, be_sh, seg_ids,
           _want_trace=False):
    x = np.ascontiguousarray(np.asarray(x, dtype=np.float32))
    seg_ids = np.asarray(seg_ids, dtype=np.int32)
    plans, max_rows = _plan(seg_ids)
    nblk = max_rows // RB

    key = (max_rows,)
    if key not in _cache:
        _cache[key] = _build(max_rows)
    nc = _cache[key]

    import ml_dtypes
    bf = ml_dtypes.bfloat16
    wfT = np.ascontiguousarray(np.asarray(W_fc, np.float32).T).astype(bf)
    wsT = np.ascontiguousarray(np.asarray(W_sh, np.float32).T).astype(bf)
    par = np.zeros((128, 162), dtype=np.float32)
    par[:, 0:8] = np.asarray(g_fc, np.float32).reshape(8, 128).T
    par[:, 8:16] = np.asarray(be_fc, np.float32).reshape(8, 128).T
    par[:, 16:24] = np.asarray(g_sh, np.float32).reshape(8, 128).T
    par[:, 24:32] = np.asarray(be_sh, np.float32).reshape(8, 128).T
    par[:, 32] = np.arange(128, dtype=np.float32)
    par[:, 34:162] = np.eye(128, dtype=np.float32)

    in_maps = []
    for c in range(NC):
        p = plans[c]
        xp = np.zeros((max_rows, D), dtype=bf)
        xp[p["col_p"]] = x[p["grows"]].astype(bf)
        # [ (kc p), (ib r) ] -> block-major [ (ib p), (kc r) ]
        xbm = np.ascontiguousarray(
            xp.T.reshape(KC, 128, nblk, RB).transpose(2, 1, 0, 3)
            .reshape(nblk * 128, KC * RB))
        in_maps.append(dict(
            xT=xbm,
            ein=p["ein"].reshape(nblk * 128, RB).astype(bf),
            widx=p["widx"],
            oh=p["oh"].reshape(nblk * 128, B_PER).astype(bf),
            wfT=wfT, wsT=wsT, par=par))

    kw = {}
    if _want_trace:
        kw = dict(trace=True)
    res = run_bass_kernel_spmd(nc, in_maps, core_ids=list(range(NC)), **kw)

    out = np.empty((N, D), dtype=np.float32)
    for c in range(NC):
        p = plans[c]
        oT = res.results[c]["outT"]          # [nblk*128, DC*RB] bf16
        # [ib, p, dc, r] -> natural [ib*RB + r, dc*128 + p]
        onat = oT.reshape(nblk, 128, DC, RB).transpose(0, 3, 2, 1) \
                 .reshape(max_rows, D)
        out[p["grows"]] = onat[p["col_p"]].astype(np.float32)
    if _want_trace:
        return out, res
    return out
